# revision 71
# baseline (speedup 1.0000x reference)
"""Trainium2 Bass kernel for a dense transformer block (B=8, S=2048, D=768, H=3072).

Sharding: pure data-parallel over batch -- one batch element per NeuronCore.

All heavy matmuls run as fp8-e4m3 with MatmulPerfMode.DoubleRow (two 128-row
contraction arms per instruction at 0.5 cycles/row -> 4x the fp32r MAC rate).
Error control (absmax gate 2e-2, measured 6.8e-3 in fp64 emulation):
  - attention path (q/k/v/scores/exp/y/o) is naive e4m3: errors average out
    over the 768/2048-long contractions (each source < 3.2e-3 alone).
  - MLP operands carry hi+lo splits: h2 = hi + lo (both e4m3), W = Whi(e4m3)
    + Wlo(e5m2, captures sub-2^-9 residuals without rescaling). Each matmul
    takes three DoubleRow passes: Whi*hi + Whi*lo + Wlo*hi.
  - exp is computed as exp(s/sqrt(d) - 3ln2) = e/8 so the UNNORMALIZED
    attention accumulator sum(e*v) stays below the e4m3 max of 240; the
    softmax 1/Z is applied per-token on the x2 write (via a PE-transposed
    reciprocal column), so o-proj never waits on the normalization chain,
    and bo enters the PSUM group as bo*Z so the 1/Z scaling cancels it.
  - weights are quantized host-side with the LN gammas folded in
    (h @ diag(g) @ W == LN_nogamma(h) @ W'), betas folded into the biases.
  - LN rstd = rsqrt(var+eps) via the quake bit-trick + Newton on DVE integer
    ALU ops, keeping the Sqrt activation table (and its 1.3us reloads
    between softmax exps) off the ACT engine entirely.

Layout (per core): feature-major hT/kT/qT [P, DT, S] fp8; token-major v
[P, ST, D] fp8; x2 kept f32 in SBUF; h2/m stored as [.., 2, ..] hi/lo pairs.
Per-feature biases (bv, bo, bproj) are added inside the PSUM group by a tiny
[1,128]x[1,D] bf16 ones-row matmul instead of an extra vector pass.

Schedule: LN1 runs in groups of 4 tiles (batched rsqrt) one group ahead of
the transpose/v/k consumers; chunk-0 scores+exp are woven into phase 1;
LN2+transposes for chunks 1-3 run inside the PE-bound MLP phase where the
vector engines idle. GPSIMD (Pool) only ever touches SBUF (hw restriction);
all PSUM reads go through DVE/ACT.
"""

import numpy as np

P = 128
S, D, H = 2048, 768, 3072
DT = D // P            # 6 d-tiles
HT = H // P            # 24 h-tiles
ST = S // P            # 16 token tiles
CH = 512               # s1 chunk width
NCH = S // CH          # 4 chunks
TPC = CH // P          # 4 token tiles per chunk
EPS = 1e-5
N_CORES = 8
LN4 = 2.0794415416798357   # 3*ln2; exp bias so e8 = exp(s)/8

WEIGHT_NAMES = [
    "ln1_g", "ln1_b", "ln2_g", "ln2_b",
    "Wq", "bq", "Wk", "bk", "Wv", "bv", "Wo", "bo",
    "Wfc", "bfc", "Wproj", "bproj",
]

_CACHE = {}


def _prep(inputs):
    """Host-side weight quantization + LN/bias folding (pure numpy)."""
    import ml_dtypes
    E4, E5, BF = ml_dtypes.float8_e4m3, ml_dtypes.float8_e5m2, ml_dtypes.bfloat16
    f32 = lambda k: np.asarray(inputs[k], dtype=np.float32)
    g1, b1 = f32("ln1_g"), f32("ln1_b")
    g2, b2 = f32("ln2_g"), f32("ln2_b")
    Wq, Wk, Wv, Wo = f32("Wq"), f32("Wk"), f32("Wv"), f32("Wo")
    Wfc, Wproj = f32("Wfc"), f32("Wproj")
    q8 = lambda a: np.ascontiguousarray(a.astype(E4))
    Wfc_g = g2[:, None] * Wfc
    Wfh = Wfc_g.astype(E4)
    Wph = Wproj.astype(E4)
    return {
        "wq8": q8(g1[:, None] * Wq), "wk8": q8(g1[:, None] * Wk),
        "wv8": q8(g1[:, None] * Wv), "wo8": q8(Wo),
        "bq_": f32("bq") + b1 @ Wq, "bk_": f32("bk") + b1 @ Wk,
        "bo16": np.asarray(
            f32("bo") + (f32("bv") + b1 @ Wv) @ Wo, dtype=BF),
        "wfh": np.ascontiguousarray(Wfh),
        "wfl": np.ascontiguousarray(
            (Wfc_g - Wfh.astype(np.float32)).astype(E5)),
        "wph": np.ascontiguousarray(Wph),
        "wpl": np.ascontiguousarray(
            (Wproj - Wph.astype(np.float32)).astype(E5)),
        "bfc_": f32("bfc") + b2 @ Wfc,
        "bp16": np.asarray(f32("bproj"), dtype=BF),
    }


def _build():
    import concourse.bass as bass
    import concourse.tile as tile
    from concourse import bacc, mybir
    from concourse.masks import make_identity
    from contextlib import ExitStack

    F = mybir.dt.float32
    BF = mybir.dt.bfloat16
    E4 = mybir.dt.float8e4
    E5 = mybir.dt.float8e5
    I32 = mybir.dt.int32
    AF = mybir.ActivationFunctionType
    OP = mybir.AluOpType
    DR = mybir.MatmulPerfMode.DoubleRow

    nc = bacc.Bacc(None, target_bir_lowering=False)

    x_d = nc.dram_tensor("x", [S, D], F, kind="ExternalInput")
    out_d = nc.dram_tensor("out", [S, D], F, kind="ExternalOutput")
    wq8_d = nc.dram_tensor("wq8", [D, D], E4, kind="ExternalInput")
    wk8_d = nc.dram_tensor("wk8", [D, D], E4, kind="ExternalInput")
    wv8_d = nc.dram_tensor("wv8", [D, D], E4, kind="ExternalInput")
    wo8_d = nc.dram_tensor("wo8", [D, D], E4, kind="ExternalInput")
    wfh_d = nc.dram_tensor("wfh", [D, H], E4, kind="ExternalInput")
    wfl_d = nc.dram_tensor("wfl", [D, H], E5, kind="ExternalInput")
    wph_d = nc.dram_tensor("wph", [H, D], E4, kind="ExternalInput")
    wpl_d = nc.dram_tensor("wpl", [H, D], E5, kind="ExternalInput")
    bq_d = nc.dram_tensor("bq_", [D], F, kind="ExternalInput")
    bk_d = nc.dram_tensor("bk_", [D], F, kind="ExternalInput")
    bfc_d = nc.dram_tensor("bfc_", [H], F, kind="ExternalInput")
    bo_d = nc.dram_tensor("bo16", [D], BF, kind="ExternalInput")
    bp_d = nc.dram_tensor("bp16", [D], BF, kind="ExternalInput")

    def bcast_ap(dram_t, n_part=P):
        ap = dram_t.ap()
        return bass.AP(tensor=ap.tensor, offset=ap.offset,
                       ap=[[0, n_part]] + list(ap.ap))

    inv_sqrt_d = 1.0 / float(np.sqrt(np.float32(D)))

    with tile.TileContext(nc) as tc, ExitStack() as ctx:
        singles = ctx.enter_context(tc.tile_pool(name="singles", bufs=1))

        ident16 = singles.tile([P, P], BF)
        make_identity(nc, ident16)
        ones_row = singles.tile([1, P], BF)
        nc.vector.memset(ones_row, 1.0)
        eps_t = singles.tile([P, 1], F)
        nc.vector.memset(eps_t, EPS)
        bo_row = singles.tile([1, D], BF)
        bp_row = singles.tile([1, D], BF)
        zero_t = singles.tile([P, 1], F)
        nc.vector.memset(zero_t, 0.0)
        bq_col = singles.tile([P, DT], F)
        bk_col = singles.tile([P, DT], F)
        bfc_col = singles.tile([P, HT], F)
        ident32 = singles.tile([P, P], F)
        make_identity(nc, ident32)
        ones8 = singles.tile([P, 2, P], E4)
        nc.vector.memset(ones8, 1.0)
        nln4_t = singles.tile([P, 1], F)
        nc.vector.memset(nln4_t, -LN4)

        # persistent activations
        perm = ctx.enter_context(tc.tile_pool(name="perm", bufs=1))
        x2_sb = perm.tile([P, ST, D], F)           # residual stream (48KB/p)
        h2s = perm.tile([P, NCH, DT, 2, CH], E4)   # LN2 out hi/lo (24KB/p)

        wmlp_ctx = ExitStack()
        wfcp = wmlp_ctx.enter_context(tc.tile_pool(name="wfcp", bufs=1))

        qkv_ctx = ExitStack()
        qkvp = qkv_ctx.enter_context(tc.tile_pool(name="qkv", bufs=1))
        k8 = qkvp.tile([P, DT, S], E4)
        q8 = qkvp.tile([P, DT, S], E4)
        v8 = qkvp.tile([P, ST, D], E4)
        wqkv = qkv_ctx.enter_context(tc.tile_pool(name="wqkv", bufs=1))
        wv_t = wqkv.tile([P, DT, D], E4)
        wk_t = wqkv.tile([P, DT, D], E4)
        wq_t = wqkv.tile([P, DT, D], E4)
        wo_t = wqkv.tile([P, DT, D], E4)

        e8_ctx = ExitStack()
        e8p = e8_ctx.enter_context(tc.tile_pool(name="e8p", bufs=2))

        # ------------- Phase 1: LN1 -> hT8; v, k, q (all fp8) -------------
        hT_ctx = ExitStack()
        hTp = hT_ctx.enter_context(tc.tile_pool(name="hT", bufs=1))
        hT8 = hTp.tile([P, DT, S], E4)

        with (
            tc.tile_pool(name="ph1", bufs=3) as ph1,
            tc.tile_pool(name="ps_tr", bufs=2, space="PSUM") as ps_trp,
            tc.tile_pool(name="ps_v", bufs=1, space="PSUM") as ps_vp,
            tc.tile_pool(name="ps_k", bufs=4, space="PSUM") as ps_kp,
        ):
            x_ts = [None] * ST
            h_ts = [None] * ST
            e8_c0 = None

            def ln1_group(g):
                mvg = ph1.tile([P, TPC, 2], F, tag="mvg")
                for i in range(TPC):
                    st = TPC * g + i
                    x_t = ph1.tile([P, D], F, tag="xt", bufs=6)
                    if st % 2:
                        nc.gpsimd.dma_start(
                            out=x_t, in_=x_d.ap()[st * P:(st + 1) * P, :])
                    else:
                        nc.sync.dma_start(x_t,
                                          x_d.ap()[st * P:(st + 1) * P, :])
                    if st == 0:
                        nc.sync.dma_start(
                            wv_t[:],
                            wv8_d.ap().rearrange("(t p) n -> p t n", p=P))
                    if st == 1:
                        nc.sync.dma_start(
                            wk_t[:],
                            wk8_d.ap().rearrange("(t p) n -> p t n", p=P))
                        nc.sync.dma_start(
                            bk_col, bk_d.ap().rearrange("(t p) -> p t", p=P))
                        nc.sync.dma_start(
                            bq_col, bq_d.ap().rearrange("(t p) -> p t", p=P))
                    if st == 2:
                        nc.sync.dma_start(
                            wq_t[:],
                            wq8_d.ap().rearrange("(t p) n -> p t n", p=P))
                    x_ts[st] = x_t
                    stats = ph1.tile([P, 3, 6], F, tag="st")
                    for j in range(3):
                        nc.vector.bn_stats(out=stats[:, j, :],
                                           in_=x_t[:, j * 256:(j + 1) * 256])
                    nc.vector.bn_aggr(out=mvg[:, i, :], in_=stats)
                # batched rsqrt(var+eps): quake bit-trick + 1 Newton (DVE)
                rsg = ph1.tile([P, TPC], F, tag="rsg")
                nc.vector.tensor_scalar(out=rsg, in0=mvg[:, :, 1], scalar1=EPS,
                                        scalar2=None, op0=OP.add)
                rig = ph1.tile([P, TPC], I32, tag="rig")
                nc.vector.tensor_scalar(out=rig, in0=rsg[:].bitcast(I32),
                                        scalar1=1, scalar2=None,
                                        op0=OP.logical_shift_right)
                nc.vector.tensor_scalar(out=rig, in0=rig, scalar1=-1,
                                        scalar2=None, op0=OP.bitwise_xor)
                nc.vector.tensor_scalar(out=rig, in0=rig, scalar1=0x5f3759e0,
                                        scalar2=None, op0=OP.add)
                rng = ph1.tile([P, TPC], F, tag="rng")
                nc.vector.tensor_tensor(out=rng, in0=rig[:].bitcast(F),
                                        in1=rig[:].bitcast(F), op=OP.mult)
                nc.vector.tensor_tensor(out=rng, in0=rng, in1=rsg, op=OP.mult)
                nc.vector.tensor_scalar(out=rng, in0=rng, scalar1=-0.5,
                                        scalar2=1.5, op0=OP.mult, op1=OP.add)
                nc.vector.tensor_tensor(out=rsg, in0=rig[:].bitcast(F),
                                        in1=rng, op=OP.mult)
                for i in range(TPC):
                    st = TPC * g + i
                    h_t = ph1.tile([P, D], BF, tag="ht", bufs=6)
                    nc.gpsimd.tensor_scalar(out=h_t, in0=x_ts[st],
                                            scalar1=mvg[:, i, 0:1],
                                            scalar2=rsg[:, i:i + 1],
                                            op0=OP.subtract, op1=OP.mult)
                    h_ts[st] = h_t

            def consume_group(pg):
                nonlocal e8_c0
                for i in range(TPC):
                    sp = TPC * pg + i
                    h_t = h_ts[sp]
                    ps_tr = ps_trp.tile([P, DT, P], BF, tag="tr")
                    for dt_ in range(DT):
                        nc.tensor.transpose(ps_tr[:, dt_, :],
                                            h_t[:, dt_ * P:(dt_ + 1) * P],
                                            ident16)
                    nc.scalar.activation(
                        out=hT8[:, :, sp * P:(sp + 1) * P],
                        in_=ps_tr, func=AF.Copy, bias=0.0, scale=1.0)
                for i in range(TPC):
                    sv = TPC * pg + i
                    ps_v = ps_vp.tile([P, 1024], F, tag="v")
                    for dc, lo, w in ((0, 0, 512), (1, 512, 256)):
                        for j in range(DT // 2):
                            nc.tensor.matmul(
                                ps_v[:, lo:lo + w],
                                hT8[:, 2 * j:2 * j + 2, sv * P:(sv + 1) * P],
                                wv_t[:, 2 * j:2 * j + 2, lo:lo + w],
                                start=(j == 0), stop=(j == DT // 2 - 1),
                                perf_mode=DR)
                    # bv is folded into bo' host-side: sum(e*(v+bv)) =
                    # sum(e*v) + Z*bv, and the Z*bv@Wo term rides the bo row
                    nc.scalar.activation(out=v8[:, sv, :], in_=ps_v[:, :D],
                                         func=AF.Copy, bias=0.0, scale=1.0)
                kc = pg
                for dtp in range(DT):
                    ps_k = ps_kp.tile([P, CH], F, tag="k")
                    for j in range(DT // 2):
                        nc.tensor.matmul(
                            ps_k,
                            wk_t[:, 2 * j:2 * j + 2, dtp * P:(dtp + 1) * P],
                            hT8[:, 2 * j:2 * j + 2, kc * CH:(kc + 1) * CH],
                            start=(j == 0), stop=(j == DT // 2 - 1),
                            perf_mode=DR)
                    if dtp % 2:
                        nc.scalar.activation(
                            out=k8[:, dtp, kc * CH:(kc + 1) * CH], in_=ps_k,
                            func=AF.Identity, bias=bk_col[:, dtp:dtp + 1],
                            scale=1.0)
                    else:
                        nc.vector.tensor_scalar(
                            out=k8[:, dtp, kc * CH:(kc + 1) * CH], in0=ps_k,
                            scalar1=bk_col[:, dtp:dtp + 1], scalar2=None,
                            op0=OP.add)
                if kc == 0:
                    e8_c0 = e8p.tile([P, ST, CH], E4, tag="e8", name="e8_c0")
                    for dtp in range(DT):
                        ps_q = ps_kp.tile([P, CH], F, tag="k", name="ps_q0")
                        for j in range(DT // 2):
                            nc.tensor.matmul(
                                ps_q,
                                wq_t[:, 2 * j:2 * j + 2,
                                     dtp * P:(dtp + 1) * P],
                                hT8[:, 2 * j:2 * j + 2, 0:CH],
                                start=(j == 0), stop=(j == DT // 2 - 1),
                                perf_mode=DR)
                        if dtp % 2:
                            nc.scalar.activation(
                                out=q8[:, dtp, 0:CH], in_=ps_q,
                                func=AF.Identity,
                                bias=bq_col[:, dtp:dtp + 1], scale=1.0)
                        else:
                            nc.vector.tensor_scalar(
                                out=q8[:, dtp, 0:CH], in0=ps_q,
                                scalar1=bq_col[:, dtp:dtp + 1], scalar2=None,
                                op0=OP.add)
                # chunk-0 scores for this k-chunk's keys
                for st2 in range(TPC * kc, TPC * kc + TPC):
                    ps_s = ps_kp.tile([P, CH], F, tag="k", name="ps_s0")
                    for j in range(DT // 2):
                        nc.tensor.matmul(
                            ps_s,
                            k8[:, 2 * j:2 * j + 2, st2 * P:(st2 + 1) * P],
                            q8[:, 2 * j:2 * j + 2, 0:CH],
                            start=(j == 0), stop=(j == DT // 2 - 1),
                            perf_mode=DR)
                    nc.scalar.activation(out=e8_c0[:, st2, :], in_=ps_s,
                                         func=AF.Exp, scale=inv_sqrt_d,
                                         bias=nln4_t)

            for g in range(TPC + 1):
                if g >= 1:
                    consume_group(g - 1)
                if g < TPC:
                    ln1_group(g)
            # q for chunks 1..3 (frees hT afterwards)
            nc.sync.dma_start(wo_t[:],
                              wo8_d.ap().rearrange("(t p) n -> p t n", p=P))
            nc.sync.dma_start(bo_row, bo_d.ap().unsqueeze(0))
            nc.sync.dma_start(bp_row, bp_d.ap().unsqueeze(0))
            nc.sync.dma_start(bfc_col,
                              bfc_d.ap().rearrange("(t p) -> p t", p=P))
            for sc in range(1, NCH):
                for dtp in range(DT):
                    ps_q = ps_kp.tile([P, CH], F, tag="k")
                    for j in range(DT // 2):
                        nc.tensor.matmul(
                            ps_q,
                            wq_t[:, 2 * j:2 * j + 2, dtp * P:(dtp + 1) * P],
                            hT8[:, 2 * j:2 * j + 2, sc * CH:(sc + 1) * CH],
                            start=(j == 0), stop=(j == DT // 2 - 1),
                            perf_mode=DR)
                    if dtp % 2:
                        nc.scalar.activation(
                            out=q8[:, dtp, sc * CH:(sc + 1) * CH], in_=ps_q,
                            func=AF.Identity, bias=bq_col[:, dtp:dtp + 1],
                            scale=1.0)
                    else:
                        nc.vector.tensor_scalar(
                            out=q8[:, dtp, sc * CH:(sc + 1) * CH], in0=ps_q,
                            scalar1=bq_col[:, dtp:dtp + 1], scalar2=None,
                            op0=OP.add)
        hT_ctx.close()

        def quake_rsqrt(pool, mvs, rss):
            # rsqrt(var+eps): quake bit-trick + 2 Newton steps, all on DVE
            vb = pool.tile([P, TPC], F, tag="vb")
            nc.vector.tensor_scalar(out=vb, in0=mvs[:, :, 1], scalar1=EPS,
                                    scalar2=None, op0=OP.add)
            ib = pool.tile([P, TPC], I32, tag="ib")
            nc.vector.tensor_scalar(out=ib, in0=vb[:].bitcast(I32),
                                    scalar1=1, scalar2=None,
                                    op0=OP.logical_shift_right)
            nc.vector.tensor_scalar(out=ib, in0=ib, scalar1=-1,
                                    scalar2=None, op0=OP.bitwise_xor)
            nc.vector.tensor_scalar(out=ib, in0=ib, scalar1=0x5f3759e0,
                                    scalar2=None, op0=OP.add)
            nc.vector.tensor_copy(out=rss, in_=ib[:].bitcast(F))
            nt = pool.tile([P, TPC], F, tag="nt")
            for _ in range(2):
                nc.vector.tensor_tensor(out=nt, in0=rss, in1=rss, op=OP.mult)
                nc.vector.tensor_tensor(out=nt, in0=nt, in1=vb, op=OP.mult)
                nc.vector.tensor_scalar(out=nt, in0=nt, scalar1=-0.5,
                                        scalar2=1.5, op0=OP.mult, op1=OP.add)
                nc.vector.tensor_tensor(out=rss, in0=rss, in1=nt, op=OP.mult)

        # ------------- Phase 2: attention + LN2 (per chunk) -------------
        with (
            tc.tile_pool(name="ph3", bufs=2) as ph3,
            tc.tile_pool(name="h2p", bufs=6) as h2p,
            tc.tile_pool(name="ytp", bufs=2) as ytp,
            tc.tile_pool(name="ps_sc", bufs=2, space="PSUM") as ps_scp,
            tc.tile_pool(name="ps_y", bufs=6, space="PSUM") as ps_yp,
        ):
            wfh_t = None
            h2_prev = None   # chunk sc-1's h2_t tiles; transposed during sc
            e8_list = [None] * NCH

            def score_exp(sc_, st2, e8_t):
                ps_s = ps_scp.tile([P, CH], F, tag="sc", name="ps_se")
                for j in range(DT // 2):
                    nc.tensor.matmul(
                        ps_s,
                        k8[:, 2 * j:2 * j + 2, st2 * P:(st2 + 1) * P],
                        q8[:, 2 * j:2 * j + 2, sc_ * CH:(sc_ + 1) * CH],
                        start=(j == 0), stop=(j == DT // 2 - 1),
                        perf_mode=DR)
                nc.scalar.activation(out=e8_t[:, st2, :], in_=ps_s,
                                     func=AF.Exp, scale=inv_sqrt_d,
                                     bias=nln4_t)

            def emit_transposes(pc, h2_ts):
                for sp in range(TPC):
                    h2_t = h2_ts[sp]
                    ps_tr = ps_yp.tile([P, DT, P], BF, tag="y",
                                       name="ps_tr2")
                    for dt_ in range(DT):
                        nc.tensor.transpose(
                            ps_tr[:, dt_, :],
                            h2_t[:, dt_ * P:(dt_ + 1) * P], ident16)
                    hi = h2s[:, pc, :, 0, sp * P:(sp + 1) * P]
                    nc.scalar.activation(out=hi, in_=ps_tr, func=AF.Copy,
                                         bias=0.0, scale=1.0)
                    nc.vector.tensor_tensor(
                        out=h2s[:, pc, :, 1, sp * P:(sp + 1) * P],
                        in0=ps_tr, in1=hi, op=OP.subtract)

            e8_list[0] = e8_c0
            for sc in range(NCH):
                e8 = e8_list[sc]
                ps_ys = [ps_yp.tile([P, CH], F, tag="y", name=f"ps_y{i}")
                         for i in range(DT)]
                for st2 in range(ST + 2):
                    # st2 0-1 of chunks 1..3 were pre-warmed by the previous
                    # chunk's tail so the first yT pair never waits on exp
                    if 2 <= st2 < ST and sc > 0:
                        score_exp(sc, st2, e8)
                    if st2 >= 2 and st2 % 2 == 0:
                        pr = st2 // 2 - 1
                        t0 = 2 * pr
                        for dtp in range(DT):
                            nc.tensor.matmul(
                                ps_ys[dtp],
                                v8[:, t0:t0 + 2, dtp * P:(dtp + 1) * P],
                                e8[:, t0:t0 + 2, :],
                                start=(pr == 0), stop=(pr == ST // 2 - 1),
                                perf_mode=DR)
                # pre-warm next chunk's exp pipeline
                if sc + 1 < NCH:
                    e8_list[sc + 1] = e8p.tile([P, ST, CH], E4, tag="e8", name="e8n")
                    for st2 in (0, 1):
                        score_exp(sc + 1, st2, e8_list[sc + 1])
                # yT stays unnormalized (values < 240 thanks to the e/8
                # scaling); 1/Z is applied per-token on the x2 write instead,
                # so o-proj never waits on the rz chain. Copies start right
                # at the pair-7 stop and drain during the Z matmuls.
                yT8 = ytp.tile([P, DT, CH], E4, tag="yt")
                for dtp in range(DT):
                    if dtp % 2:
                        nc.scalar.activation(out=yT8[:, dtp], in_=ps_ys[dtp],
                                             func=AF.Copy, bias=0.0, scale=1.0)
                    else:
                        nc.vector.tensor_copy(out=yT8[:, dtp], in_=ps_ys[dtp])
                # Z after the exps, in the scores slot rotation
                ps_z = ps_scp.tile([P, CH], F, tag="sc", name="ps_z")
                for pr in range(ST // 2):
                    nc.tensor.matmul(ps_z, ones8, e8[:, 2 * pr:2 * pr + 2, :],
                                     start=(pr == 0), stop=(pr == ST // 2 - 1),
                                     perf_mode=DR)
                z_row = ph3.tile([1, CH], BF, tag="zrow")
                nc.vector.tensor_copy(out=z_row, in_=ps_z[0:1, :])
                rz = ph3.tile([P, CH], F, tag="rz")
                nc.vector.reciprocal(out=rz, in_=ps_z)
                # transpose rz into per-token columns for the x2 scaling
                ps_rzt = ps_yp.tile([P, TPC, P], F, tag="y", name="ps_rzt")
                for su in range(TPC):
                    nc.tensor.transpose(ps_rzt[:, su, :],
                                        rz[:, su * P:(su + 1) * P], ident32)
                rz_cols = ph3.tile([P, TPC], F, tag="rzc")
                nc.vector.tensor_copy(out=rz_cols, in_=ps_rzt[:, :, 0:1])
                h2_ts = [None] * TPC
                mvs = ph3.tile([P, TPC, 2], F, tag="mvs")
                rss = ph3.tile([P, TPC], F, tag="rss")
                for su in range(TPC):
                    st = sc * TPC + su
                    x_t = ph3.tile([P, D], F, tag="xt3")
                    nc.sync.dma_start(x_t, x_d.ap()[st * P:(st + 1) * P, :])
                    ps_o0 = ps_scp.tile([P, CH], F, tag="sc", name="ps_o0")
                    ps_o1 = ps_yp.tile([P, CH], F, tag="y", name="ps_o1")
                    for ps_o, lo, w in ((ps_o0, 0, 512), (ps_o1, 512, 256)):
                        for j in range(DT // 2):
                            nc.tensor.matmul(
                                ps_o[:, :w],
                                yT8[:, 2 * j:2 * j + 2, su * P:(su + 1) * P],
                                wo_t[:, 2 * j:2 * j + 2, lo:lo + w],
                                start=(j == 0), stop=False, perf_mode=DR)
                        # bo enters as bo*Z so the 1/Z scaling cancels it out
                        nc.tensor.matmul(ps_o[:, :w],
                                         z_row[:, su * P:(su + 1) * P],
                                         bo_row[:, lo:lo + w],
                                         start=False, stop=True)
                    nc.vector.tensor_scalar(out=x2_sb[:, st, :512],
                                            in0=ps_o0,
                                            scalar1=rz_cols[:, su:su + 1],
                                            scalar2=None, op0=OP.mult)
                    nc.gpsimd.tensor_tensor(out=x2_sb[:, st, :512],
                                            in0=x2_sb[:, st, :512],
                                            in1=x_t[:, :512], op=OP.add)
                    nc.vector.tensor_scalar(out=x2_sb[:, st, 512:],
                                            in0=ps_o1[:, :256],
                                            scalar1=rz_cols[:, su:su + 1],
                                            scalar2=None, op0=OP.mult)
                    nc.gpsimd.tensor_tensor(out=x2_sb[:, st, 512:],
                                            in0=x2_sb[:, st, 512:],
                                            in1=x_t[:, 512:], op=OP.add)
                    if sc == 0:
                        # LN2 stats for chunk 0 only; later chunks' LN2 runs
                        # inside the PE-bound MLP phase where engines idle
                        stats = ph3.tile([P, 3, 6], F, tag="st3")
                        for i in range(3):
                            nc.vector.bn_stats(out=stats[:, i, :],
                                               in_=x2_sb[:, st,
                                                         i * 256:(i + 1) * 256])
                        nc.vector.bn_aggr(out=mvs[:, su, :], in_=stats)
                if sc == 0:
                    quake_rsqrt(ph3, mvs, rss)
                    for su in range(TPC):
                        st = sc * TPC + su
                        h2_t = h2p.tile([P, D], BF, tag="h2")
                        nc.gpsimd.tensor_scalar(out=h2_t, in0=x2_sb[:, st, :],
                                                scalar1=mvs[:, su, 0:1],
                                                scalar2=rss[:, su:su + 1],
                                                op0=OP.subtract, op1=OP.mult)
                        h2_ts[su] = h2_t
                    h2_prev = h2_ts
                if sc == 1:
                    emit_transposes(0, h2_prev)
                # prefetch MLP fc hi-weights while attention runs
                if sc == 0:
                    wfh_t = wfcp.tile([P, DT, H], E4)
                    nc.sync.dma_start(
                        wfh_t[:], wfh_d.ap().rearrange("(t p) n -> p t n", p=P))
        e8_ctx.close()
        qkv_ctx.close()

        # ------------- Phase 3: MLP (per chunk) -------------
        wprp = wmlp_ctx.enter_context(tc.tile_pool(name="wprp", bufs=1))
        wfl_t = wprp.tile([P, DT, H], E5)
        for pc in range(3):
            lo, hi = pc * (H // 3), (pc + 1) * (H // 3)
            nc.sync.dma_start(
                wfl_t[:, :, lo:hi],
                wfl_d.ap()[:, lo:hi].rearrange("(t p) n -> p t n", p=P))
        wph_t = wprp.tile([P, HT, D], E4)
        nc.sync.dma_start(wph_t[:],
                          wph_d.ap().rearrange("(t p) n -> p t n", p=P))
        wpl_t = wprp.tile([P, HT, D], E5)
        nc.sync.dma_start(wpl_t[:],
                          wpl_d.ap().rearrange("(t p) n -> p t n", p=P))
        with (
            tc.tile_pool(name="ph5", bufs=3) as ph5,
            tc.tile_pool(name="msp", bufs=1) as msp,
            tc.tile_pool(name="ps_u", bufs=3, space="PSUM") as ps_up,
            tc.tile_pool(name="ps_tr3", bufs=1, space="PSUM") as ps_tr3p,
            tc.tile_pool(name="ps_o2", bufs=2, space="PSUM") as ps_o2p,
        ):
            ms = msp.tile([P, HT, 2, CH], E4)

            def mlp_ln2(pc):
                # LN2 + transposes + hi/lo split for chunk pc, overlapped
                # with the PE-bound fc/proj stream
                mvs3 = ph5.tile([P, TPC, 2], F, tag="mvs3")
                rss3 = ph5.tile([P, TPC], F, tag="rss3")
                for su in range(TPC):
                    st = pc * TPC + su
                    stats = ph5.tile([P, 3, 6], F, tag="st5")
                    for i in range(3):
                        nc.vector.bn_stats(out=stats[:, i, :],
                                           in_=x2_sb[:, st,
                                                     i * 256:(i + 1) * 256])
                    nc.vector.bn_aggr(out=mvs3[:, su, :], in_=stats)
                quake_rsqrt(ph5, mvs3, rss3)
                for su in range(TPC):
                    st = pc * TPC + su
                    h2_t = ph5.tile([P, D], BF, tag="h2m", bufs=4)
                    nc.vector.tensor_scalar(out=h2_t, in0=x2_sb[:, st, :],
                                            scalar1=mvs3[:, su, 0:1],
                                            scalar2=rss3[:, su:su + 1],
                                            op0=OP.subtract, op1=OP.mult)
                    ps_tr = ps_tr3p.tile([P, DT, P], BF, tag="tr3")
                    for dt_ in range(DT):
                        nc.tensor.transpose(ps_tr[:, dt_, :],
                                            h2_t[:, dt_ * P:(dt_ + 1) * P],
                                            ident16)
                    hi = h2s[:, pc, :, 0, su * P:(su + 1) * P]
                    nc.scalar.activation(out=hi, in_=ps_tr, func=AF.Copy,
                                         bias=0.0, scale=1.0)
                    nc.vector.tensor_tensor(
                        out=h2s[:, pc, :, 1, su * P:(su + 1) * P],
                        in0=ps_tr, in1=hi, op=OP.subtract)

            for sc in range(NCH):
                hs = h2s[:, sc]
                for ht in range(HT):
                    ps_u = ps_up.tile([P, CH], F, tag="u")
                    hsl = ht * P
                    for j in range(DT // 2):
                        nc.tensor.matmul(
                            ps_u, wfh_t[:, 2 * j:2 * j + 2, hsl:hsl + P],
                            hs[:, 2 * j:2 * j + 2, 0, :],
                            start=(j == 0), stop=False, perf_mode=DR)
                    for j in range(DT // 2):
                        nc.tensor.matmul(
                            ps_u, wfh_t[:, 2 * j:2 * j + 2, hsl:hsl + P],
                            hs[:, 2 * j:2 * j + 2, 1, :],
                            start=False, stop=False, perf_mode=DR)
                    for j in range(DT // 2):
                        nc.tensor.matmul(
                            ps_u, wfl_t[:, 2 * j:2 * j + 2, hsl:hsl + P],
                            hs[:, 2 * j:2 * j + 2, 0, :],
                            start=False, stop=(j == DT // 2 - 1), perf_mode=DR)
                    m16 = ph5.tile([P, CH], BF, tag="m16")
                    nc.scalar.activation(out=m16, in_=ps_u, func=AF.Gelu,
                                         bias=bfc_col[:, ht:ht + 1], scale=1.0)
                    nc.gpsimd.tensor_copy(out=ms[:, ht, 0, :], in_=m16)
                    nc.gpsimd.tensor_tensor(out=ms[:, ht, 1, :], in0=m16,
                                            in1=ms[:, ht, 0, :],
                                            op=OP.subtract)
                if sc + 1 < NCH:
                    mlp_ln2(sc + 1)
                for su in range(TPC):
                    st = sc * TPC + su
                    ps_o2 = ps_o2p.tile([P, 1024], F, tag="o2")
                    for lo, w in ((0, 512), (512, 256)):
                        for arm in range(3):   # Whi*hi, Whi*lo, Wlo*hi
                            wt = wph_t if arm < 2 else wpl_t
                            mslot = 0 if arm != 1 else 1
                            for j in range(HT // 2):
                                nc.tensor.matmul(
                                    ps_o2[:, lo:lo + w],
                                    ms[:, 2 * j:2 * j + 2, mslot,
                                       su * P:(su + 1) * P],
                                    wt[:, 2 * j:2 * j + 2, lo:lo + w],
                                    start=(arm == 0 and j == 0), stop=False,
                                    perf_mode=DR)
                        nc.tensor.matmul(ps_o2[:, lo:lo + w], ones_row,
                                         bp_row[:, lo:lo + w],
                                         start=False, stop=True)
                    o_t = ph5.tile([P, D], F, tag="ot")
                    nc.vector.tensor_tensor(out=o_t[:, :512],
                                            in0=ps_o2[:, :512],
                                            in1=x2_sb[:, st, :512], op=OP.add)
                    nc.vector.tensor_tensor(out=o_t[:, 512:],
                                            in0=ps_o2[:, 512:768],
                                            in1=x2_sb[:, st, 512:], op=OP.add)
                    nc.sync.dma_start(out_d.ap()[st * P:(st + 1) * P, :], o_t)
        wmlp_ctx.close()

    return nc


def _get_nc():
    if "nc" not in _CACHE:
        nc = _build()
        nc.compile()
        _CACHE["nc"] = nc
    return _CACHE["nc"]


TRACE = False


def kernel(**inputs):
    from concourse.bass_utils import run_bass_kernel_spmd

    nc = _get_nc()
    x = np.asarray(inputs["x"], dtype=np.float32)
    base = _prep(inputs)
    in_maps = [dict(base, x=np.ascontiguousarray(x[b])) for b in range(N_CORES)]
    res = run_bass_kernel_spmd(nc, in_maps, core_ids=list(range(N_CORES)),
                               trace=TRACE)
    _CACHE["last_res"] = res
    return np.stack([res.results[b]["out"] for b in range(N_CORES)], axis=0)
